# revision 1
# baseline (speedup 1.0000x reference)
"""Trainium2 Bass kernel for CrossAttnMem (q-augmented cross attention with
a shared K/V memory bank, InstanceNorm'd scores, softmax over the bank axis).

Sharding: 8 query batches over 8 cores; each core holds its query slice plus
the full emb_u (replicated) and computes its output slice locally (no
collectives).

The K/V bank is rank-64 (a projection of the 64-channel emb_u), so every
bank-sized contraction is reassociated through the embedding dimension:

    M^T[b]  = emb_l^T @ emb_u[b]            [64, 64]   (K = n)
    G[b]    = (M^T[b])^T @ Wq^T             [64, CH]   (== emb_u[b]^T @ q)
    sT[b]   = Wk @ G[b]                     [CH, CH]   (scores, d-major)
    stats:    sum(s)  = sum_b <M^T[b], wq_rowsum x wk_colsum>
              sum(s^2)= sum_b <K2, M^T[b]^T Q2 M^T[b]>,  Q2 = Wq^T Wq,
                                                         K2 = Wk^T Wk
              (exact InstanceNorm stats via 64x64 trace identities)
    eT[b]   = exp(scale * sT[b] + bias)     (sT recomputed; exp straight from
                                             PSUM; no max needed: |s_n|<~8)
    H'[b]   = [Wv | 1]^T @ eT[b]            [65, CH]   (row 64 = Z_b)
    U(ns)  += H[b]^T-slices @ emb_u[b]^T    [CH, N]    (K = e, per n-half)
    Z       = sum_b H'[b][64]               -> transpose -> 1/Z per c
    out     = (U / Z)^T @ Wo^T

This cuts the bank-sized matmuls (N x CH x CH scores, CH x B*CH x N context)
down to rank-64 chains. All matmuls take fp16 inputs (full PE rate) with
fp32 PSUM accumulation; the stats path runs in fp32. Measured end-to-end
relative error vs the fp32 reference: ~6e-4.
"""

import os
import sys
from contextlib import ExitStack

import numpy as np

try:
    import concourse.bass as bass
except ImportError:  # pragma: no cover
    sys.path.insert(0, "/opt/trn_rl_repo")
    import concourse.bass as bass

import concourse.tile as tile
from concourse import bacc, mybir
from concourse.bass_utils import run_bass_kernel_spmd
from concourse.masks import make_identity

P = 128
N = 1024            # sequence length
E = 64              # embedding channels
CH = 512            # attention channels (num_heads * emb_ch)
B = 8               # kv batches (== upper half of emb batch)
NCORES = 8
NCH = N // P        # 8
CCH = CH // P       # 4
DCH = (B * CH) // P  # 32 d-chunks of the bank axis
EPS = 1e-5
NELEM = float(CH * B * CH)   # elements in one instance-norm plane

F32 = mybir.dt.float32
F16 = mybir.dt.float16
Exp = mybir.ActivationFunctionType.Exp
Sqrt = mybir.ActivationFunctionType.Sqrt
Identity = mybir.ActivationFunctionType.Identity
Mult = mybir.AluOpType.mult
Add = mybir.AluOpType.add
AxX = mybir.AxisListType.X


def build_kernel() -> bass.Bass:
    nc = bacc.Bacc("TRN2", debug=False, num_devices=NCORES)

    emb_l = nc.dram_tensor("emb_l", [N, E], F32, kind="ExternalInput")
    emb_u = nc.dram_tensor("emb_u", [B, N, E], F32, kind="ExternalInput")
    Wq = nc.dram_tensor("Wq", [CH, E], F32, kind="ExternalInput")
    Wk = nc.dram_tensor("Wk", [CH, E], F32, kind="ExternalInput")
    Wv = nc.dram_tensor("Wv", [CH, E], F32, kind="ExternalInput")
    Wo = nc.dram_tensor("Wo", [E, CH], F32, kind="ExternalInput")
    out = nc.dram_tensor("out", [N, E], F32, kind="ExternalOutput")

    with tile.TileContext(nc) as tc:
        _body(tc, emb_l, emb_u, Wq, Wk, Wv, Wo, out)
    nc.compile()
    return nc


def _body(tc, emb_l, emb_u, Wq, Wk, Wv, Wo, out):
    nc = tc.nc

    with ExitStack() as ctx:
        const = ctx.enter_context(tc.tile_pool(name="const", bufs=1))
        wpool = ctx.enter_context(tc.tile_pool(name="wpool", bufs=1))
        big = ctx.enter_context(tc.tile_pool(name="big", bufs=1))
        stream = ctx.enter_context(tc.tile_pool(name="stream", bufs=3))
        small = ctx.enter_context(tc.tile_pool(name="small", bufs=1))
        psum = ctx.enter_context(tc.tile_pool(name="psum", bufs=1, space="PSUM"))

        # PSUM budget (8 banks x 2KB/partition):
        #   tag "u": 2 x [128,2,512] f32 = 4 banks (stats 64x64s in pass A;
        #            the two U-accumulator pairs afterwards)
        #   tag "v": 1 x [128,2,512] f32 = 2 banks (G in pass A, sT recompute)
        #   tag "o": 1 x [65,512] f32   = 1 bank  (M^T, H', out projection)
        #   tag "z": 1 x <=2KB          = 1 bank  (transposes, stats, 1/Z)

        ident = const.tile([P, P], F32)
        make_identity(nc, ident)
        ident16 = const.tile([P, P], F16)
        nc.vector.tensor_copy(ident16[:], ident[:])

        # ---- weights ----
        # Wq^T, Wk^T: [E, CH] fp16 via PE transpose; keep fp16 naturals too
        wT = {}
        w16 = {}
        for wname, W in (("q", Wq), ("k", Wk)):
            w_nat = stream.tile([P, CCH, E], F32, tag="w_nat", bufs=2,
                                name=f"w_nat_{wname}")
            nc.sync.dma_start(w_nat[:], W.rearrange("(o p) e -> p o e", p=P))
            ps_wt = psum.tile([E, CH], F32, tag="z", bufs=1, name=f"ps_wt_{wname}")
            for o in range(CCH):
                nc.tensor.transpose(ps_wt[:, o * P:(o + 1) * P], w_nat[:, o, :],
                                    ident)
            wt = wpool.tile([E, CH], F16, name=f"wT_{wname}")
            nc.scalar.copy(wt[:], ps_wt[:])
            wT[wname] = wt
            wn16 = wpool.tile([P, CCH, E], F16, name=f"w16_{wname}")
            nc.vector.tensor_copy(wn16[:], w_nat[:])
            w16[wname] = wn16

        # Wv stays natural, augmented with a ones column (row 64 of H' = Z_b)
        w_natv = stream.tile([P, CCH, E], F32, tag="w_nat", bufs=2, name="w_natv")
        nc.sync.dma_start(w_natv[:], Wv.rearrange("(o p) e -> p o e", p=P))
        w_aug = wpool.tile([P, CCH, E + 1], F16, name="w_aug")
        nc.vector.tensor_copy(w_aug[:, :, 0:E], w_natv[:])
        nc.vector.memset(w_aug[:, :, E:E + 1], 1.0)

        # Wo^T: [E, CH] -> [CH, E] as [P, CCH, E] fp16
        wo_nat = stream.tile([E, CH], F32, tag="wo_nat", bufs=1, name="wo_nat")
        nc.sync.dma_start(wo_nat[:], Wo[:, :])
        ps_wo = psum.tile([P, CCH, E], F32, tag="z", bufs=1, name="ps_wo")
        for o in range(CCH):
            nc.tensor.transpose(ps_wo[:, o, :], wo_nat[:, o * P:(o + 1) * P],
                                ident[:E, :E])
        woT = wpool.tile([P, CCH, E], F16, name="woT")
        nc.scalar.copy(woT[:], ps_wo[:])

        # ---- stats constants: Q2 = Wq^T Wq, K2 = Wk^T Wk, W2 = outer sums --
        q2_sb = wpool.tile([E, E], F32, name="q2_sb")
        k2_sb = wpool.tile([E, E], F32, name="k2_sb")
        for wname, dst in (("q", q2_sb), ("k", k2_sb)):
            ps_w2m = psum.tile([E, E], F32, tag="u", bufs=2, name="ps_w2m")
            for o in range(CCH):
                nc.tensor.matmul(ps_w2m[:], w16[wname][:, o, :],
                                 w16[wname][:, o, :],
                                 start=(o == 0), stop=(o == CCH - 1))
            nc.vector.tensor_copy(dst[:], ps_w2m[:])
        # row sums of Wq^T / Wk^T over the CH axis
        wsum = small.tile([E, 2], F32, name="wsum")
        nc.vector.reduce_sum(wsum[:, 0:1], wT["q"][:], axis=AxX)
        nc.vector.reduce_sum(wsum[:, 1:2], wT["k"][:], axis=AxX)
        ps_wr = psum.tile([1, 2 * E], F32, tag="z", bufs=1, name="ps_wr")
        nc.tensor.transpose(ps_wr[:, 0:E], wsum[:, 0:1], ident[:E, :E])
        nc.tensor.transpose(ps_wr[:, E:2 * E], wsum[:, 1:2], ident[:E, :E])
        wr_sb = small.tile([1, 2 * E], F32, name="wr_sb")
        nc.vector.tensor_copy(wr_sb[:], ps_wr[:])
        ps_w2 = psum.tile([E, E], F32, tag="z", bufs=1, name="ps_w2")
        nc.tensor.matmul(ps_w2[:], wr_sb[:, 0:E], wr_sb[:, E:2 * E],
                         start=True, stop=True)
        w2_sb = wpool.tile([E, E], F32, name="w2_sb")
        nc.vector.tensor_copy(w2_sb[:], ps_w2[:])

        # ---- emb_l (natural, fp16) ----
        el_nat = stream.tile([P, NCH, E], F32, tag="emb_nat", bufs=2, name="el_nat")
        nc.sync.dma_start(el_nat[:], emb_l.rearrange("(nc p) e -> p nc e", p=P))
        el16 = wpool.tile([P, NCH, E], F16, name="el16")
        nc.vector.tensor_copy(el16[:], el_nat[:])

        # ---- pass A: per kv batch -> M^T, G, stats partials, emb_u^T ----
        euT = wpool.tile([E, B, N], F16, name="euT")
        G_sb = big.tile([E, B, CH], F16, name="G_sb")
        part_s = big.tile([E, B, 2], F32, name="part_s")
        for b in range(B):
            eu_nat = stream.tile([P, NCH, E], F32, tag="emb_nat", bufs=2,
                                 name="eu_nat")
            nc.sync.dma_start(eu_nat[:],
                              emb_u[b].rearrange("(nc p) e -> p nc e", p=P))
            eu16 = stream.tile([P, NCH, E], F16, tag="eu16", bufs=3, name="eu16")
            nc.vector.tensor_copy(eu16[:], eu_nat[:])

            # M^T[b] = emb_l^T @ emb_u[b]   [64, 64]
            ps_m = psum.tile([E, E], F32, tag="o", bufs=1, name="ps_m")
            for nch in range(NCH):
                nc.tensor.matmul(ps_m[:], el16[:, nch, :], eu16[:, nch, :],
                                 start=(nch == 0), stop=(nch == NCH - 1))
            m16 = stream.tile([E, E], F16, tag="m16", bufs=2, name="m16")
            nc.scalar.copy(m16[:], ps_m[:])
            m32 = stream.tile([E, E], F32, tag="m32", bufs=2, name="m32")
            nc.vector.tensor_copy(m32[:], ps_m[:])

            # G[b] = (M^T)^T @ Wq^T   [64, CH]
            ps_gg = psum.tile([E, CH], F32, tag="v", bufs=2, name="ps_gg")
            nc.tensor.matmul(ps_gg[:], m16[:], wT["q"][:], start=True, stop=True)
            nc.scalar.copy(G_sb[:, b, :], ps_gg[:])

            # stats partials: sum(s) via <M^T, W2>; sum(s^2) via <K2, P3>
            scr = stream.tile([E, E], F32, tag="scr", bufs=2, name="scr")
            nc.vector.tensor_mul(scr[:], m32[:], w2_sb[:])
            nc.vector.reduce_sum(part_s[:, b, 0:1], scr[:], axis=AxX)
            ps_p1 = psum.tile([E, E], F32, tag="u", bufs=2, name="ps_p1")
            nc.tensor.matmul(ps_p1[:], q2_sb[:], m32[:], start=True, stop=True)
            p1_sb = stream.tile([E, E], F32, tag="p1_sb", bufs=2, name="p1_sb")
            nc.scalar.copy(p1_sb[:], ps_p1[:])
            ps_p3 = psum.tile([E, E], F32, tag="u", bufs=2, name="ps_p3")
            nc.tensor.matmul(ps_p3[:], m32[:], p1_sb[:], start=True, stop=True)
            scr2 = stream.tile([E, E], F32, tag="scr2", bufs=2, name="scr2")
            nc.vector.tensor_mul(scr2[:], k2_sb[:], ps_p3[:])
            nc.vector.reduce_sum(part_s[:, b, 1:2], scr2[:], axis=AxX)

            # emb_u[b]^T (fp16), for the U contraction later
            for h in range(2):
                ps_et = psum.tile([E, 512], F16, tag="z", bufs=1, name="ps_eut")
                for j in range(4):
                    nch = h * 4 + j
                    nc.tensor.transpose(ps_et[:, j * P:(j + 1) * P],
                                        eu16[:, nch, :], ident16)
                if h == 0:
                    nc.vector.tensor_copy(euT[:, b, 0:512], ps_et[:])
                else:
                    nc.scalar.copy(euT[:, b, 512:1024], ps_et[:])

        # ---- global instance-norm stats ----
        ones_f = const.tile([P, 1], F32)
        nc.vector.memset(ones_f, 1.0)
        ones_row = const.tile([1, P], F32)
        nc.vector.memset(ones_row, 1.0)
        psums2 = small.tile([E, 2], F32, name="psums2")
        nc.vector.reduce_sum(psums2[:, 0:1], part_s[:, :, 0], axis=AxX)
        nc.vector.reduce_sum(psums2[:, 1:2], part_s[:, :, 1], axis=AxX)
        ps_g = psum.tile([1, 2], F32, tag="z", bufs=1, name="ps_g")
        nc.tensor.matmul(ps_g[:], ones_f[:E, :], psums2[:], start=True, stop=True)
        gm = small.tile([1, 2], F32, name="gm")
        nc.vector.tensor_scalar_mul(gm[:], ps_g[:], 1.0 / NELEM)  # [mu, E[s^2]]
        var = small.tile([1, 1], F32, name="var")
        nc.vector.tensor_mul(var[:], gm[:, 0:1], gm[:, 0:1])
        nc.vector.tensor_sub(var[:], gm[:, 1:2], var[:])
        sc = small.tile([1, 2], F32, name="sc")
        eps_t = small.tile([1, 1], F32, name="eps_t")
        nc.vector.memset(eps_t[:], EPS)
        nc.scalar.activation(sc[:, 0:1], var[:], Sqrt, bias=eps_t[:], scale=1.0)
        nc.vector.reciprocal(sc[:, 0:1], sc[:, 0:1])
        nc.vector.tensor_scalar(sc[:, 1:2], gm[:, 0:1], sc[:, 0:1], -1.0,
                                Mult, Mult)
        ps_bc = psum.tile([P, 2], F32, tag="z", bufs=1, name="ps_bc")
        nc.tensor.matmul(ps_bc[:], ones_row[:], sc[:], start=True, stop=True)
        sb_b = small.tile([P, 2], F32, name="sb_b")
        nc.vector.tensor_copy(sb_b[:], ps_bc[:])

        # ---- pass 0a: recompute sT -> exp -> H' (+Z row); U(ns=0, cc 0-1) ----
        H_sb = big.tile([E + 1, B, CH], F16, name="H_sb")
        ctx_bf = big.tile([P, CCH, N], F16, name="ctx_bf")
        out_sb = big.tile([P, NCH, E], F32, name="out_sb")
        # Z = sum_d eT[d, :] accumulates as a [1, CH] row (ones stationary)
        ones_16 = const.tile([P, 1], F16)
        nc.vector.memset(ones_16, 1.0)
        ps_z1 = psum.tile([1, CH], F32, tag="z", bufs=1, name="ps_z1")
        ups_g0 = [psum.tile([P, 512], F32, tag="u", bufs=2, name=f"ups_g0_{i}")
                  for i in range(2)]
        for b in range(B):
            eT_b = stream.tile([P, CCH, CH], F16, tag="eT_b", bufs=3, name="eT_b")
            for hp in range(2):
                ps_sb = psum.tile([P, 2, CH], F32, tag="v", bufs=2, name="ps_sb")
                for j in range(2):
                    cp = hp * 2 + j
                    nc.tensor.matmul(ps_sb[:, j, :],
                                     wT["k"][:, cp * P:(cp + 1) * P],
                                     G_sb[:, b, :], start=True, stop=True)
                nc.scalar.activation(eT_b[:, hp * 2:hp * 2 + 2, :], ps_sb[:],
                                     Exp, bias=sb_b[:, 1:2], scale=sb_b[:, 0:1])

            # H'[b] = [Wv | 1]^T @ eT[b]   [65, CH]; row 64 = Z_b
            ps_h = psum.tile([E + 1, CH], F32, tag="o", bufs=1, name="ps_h")
            for cp in range(CCH):
                nc.tensor.matmul(ps_h[:], w_aug[:, cp, :], eT_b[:, cp, :],
                                 start=(cp == 0), stop=(cp == CCH - 1))
            nc.vector.tensor_copy(H_sb[:, b, :], ps_h[:])
            for cp in range(CCH):
                nc.tensor.matmul(ps_z1[:], ones_16[:], eT_b[:, cp, :],
                                 start=(b == 0 and cp == 0),
                                 stop=(b == B - 1 and cp == CCH - 1))

            for cc in range(2):
                nc.tensor.matmul(ups_g0[cc][:],
                                 H_sb[0:E, b, cc * P:(cc + 1) * P],
                                 euT[:, b, 0:512],
                                 start=(b == 0), stop=(b == B - 1))

        # scatter Z row to c-partitions via K=1 matmuls, then invert
        z1_sb = small.tile([1, CH], F32, name="z1_sb")
        nc.vector.tensor_copy(z1_sb[:], ps_z1[:])
        ps_zt = psum.tile([P, CCH], F32, tag="z", bufs=1, name="ps_zt")
        for cc in range(CCH):
            nc.tensor.matmul(ps_zt[:, cc:cc + 1],
                             z1_sb[:, cc * P:(cc + 1) * P],
                             ident[0:1, 0:1],
                             start=(cc == 0), stop=(cc == CCH - 1))
        zr = small.tile([P, CCH], F32, name="zr")
        nc.vector.reciprocal(zr[:], ps_zt[:])

        def u_streak(ccs, ns):
            ups = [psum.tile([P, 512], F32, tag="u", bufs=2,
                             name=f"ups_{ns}_{cc}") for cc in ccs]
            for b in range(B):
                for i, cc in enumerate(ccs):
                    nc.tensor.matmul(ups[i][:],
                                     H_sb[0:E, b, cc * P:(cc + 1) * P],
                                     euT[:, b, ns * 512:(ns + 1) * 512],
                                     start=(b == 0), stop=(b == B - 1))
            return ups

        def ctx_div(ups, ccs, ns):
            for i, cc in enumerate(ccs):
                nc.vector.tensor_scalar_mul(
                    ctx_bf[:, cc, ns * 512:(ns + 1) * 512],
                    ups[i][:], zr[:, cc:cc + 1])

        def out_proj(ns):
            for j in range(4):
                nch = ns * 4 + j
                ps_o = psum.tile([P, E], F32, tag="v", bufs=2, name="ps_o")
                for cc in range(CCH):
                    nc.tensor.matmul(ps_o[:],
                                     ctx_bf[:, cc, nch * P:(nch + 1) * P],
                                     woT[:, cc, :],
                                     start=(cc == 0), stop=(cc == CCH - 1))
                if j % 2 == 0:
                    nc.scalar.copy(out_sb[:, nch, :], ps_o[:])
                else:
                    nc.vector.tensor_copy(out_sb[:, nch, :], ps_o[:])

        # ---- remaining U accumulations are pure matmul streaks ----
        ctx_div(ups_g0, (0, 1), 0)
        ups_g1 = u_streak((2, 3), 0)
        ctx_div(ups_g1, (2, 3), 0)
        out_proj(0)
        ups1_g0 = u_streak((0, 1), 1)
        ctx_div(ups1_g0, (0, 1), 1)
        ups1_g1 = u_streak((2, 3), 1)
        ctx_div(ups1_g1, (2, 3), 1)
        out_proj(1)

        nc.sync.dma_start(out.rearrange("(nc p) e -> p nc e", p=P), out_sb[:])


_NC_CACHE = None


def _get_nc():
    global _NC_CACHE
    if _NC_CACHE is None:
        _NC_CACHE = build_kernel()
    return _NC_CACHE


def kernel(**inputs) -> np.ndarray:
    emb = np.ascontiguousarray(np.asarray(inputs["emb"], dtype=np.float32))
    Wq = np.ascontiguousarray(np.asarray(inputs["Wq"], dtype=np.float32))
    Wk = np.ascontiguousarray(np.asarray(inputs["Wk"], dtype=np.float32))
    Wv = np.ascontiguousarray(np.asarray(inputs["Wv"], dtype=np.float32))
    Wo = np.ascontiguousarray(np.asarray(inputs["Wo"], dtype=np.float32))

    emb_u = np.ascontiguousarray(emb[:B])      # replicated K/V source
    in_maps = []
    for core in range(NCORES):
        in_maps.append({
            "emb_l": np.ascontiguousarray(emb[B + core]),
            "emb_u": emb_u,
            "Wq": Wq, "Wk": Wk, "Wv": Wv, "Wo": Wo,
        })

    nc = _get_nc()
    res = run_bass_kernel_spmd(nc, in_maps, core_ids=list(range(NCORES)))
    out = np.stack([res.results[c]["out"] for c in range(NCORES)], axis=0)
    return out.astype(np.float32)


if __name__ == "__main__":
    nc = build_kernel()
    print("built ok")



# revision 39
# speedup vs baseline: 1.4298x; 1.4298x over previous
"""Trainium2 Bass kernel for CrossAttnMem (q-augmented cross attention with
a shared K/V memory bank, InstanceNorm'd scores, softmax over the bank axis).

Sharding: 8 query batches over 8 cores; each core holds its query slice plus
the full emb_u (replicated) and computes its output slice locally.

Rank-64 reassociation (per kv batch b):
    M[b]   = emb_l^T @ emb_u[b]                  [64, 64]   (K = n)
    G[b]   = M[b]^T @ Wq^T                       [64, CH]
    sT[b]  = Wk @ G[b]                           [CH, CH]   (d-major scores)
    stats:   sum(s) = <wrk, sum_b rowsum(G[b])>,
             sum(s^2) = <K2, sum_b M^T Q2 M>     (exact InstanceNorm stats)
    eT[b]  = exp(s/sigma - mu/sigma)  (fp16; fp8 tested: softmax too
             concentrated after norm, quant noise -> ~3e-2 rel err)
    H'[b]  = [Wv | 1]^T @ eT[b]                  [65, CH]  (row 64 = Z_b)
    Z      = ones^T @ (Z_b rows gathered to partitions via SBUF DMA)
    U      = H2^T-slices @ euT (b-pairs packed along K=128)   [CH, N]
    out^T  = (Wo*4096/Z)^T @ (U/4096)            [64, N] -> transpose -> out

Schedule notes: single-queue DMA train (flat 2KB/partition layouts, row
n = 8p + x; weight rows c = 4p + o; Wo columns pre-permuted on host to
match); pass 0a is software-pipelined (sT(g+1) before H'(g)) so the 16 exp
activations run back-to-back on ACT; emb_u^T and H are partition-packed in
b-pairs (via transpose offsets / identity-selection matmuls) so the U
contraction runs at full K=128; one U tile accumulates inside pass 0a; a
single activation table (ln+exp+copy) is loaded once at startup.
"""

import sys
from contextlib import ExitStack

import numpy as np

try:
    import concourse.bass as bass
except ImportError:  # pragma: no cover
    sys.path.insert(0, "/opt/trn_rl_repo")
    import concourse.bass as bass

import concourse.tile as tile
from concourse import bacc, mybir
from concourse.bass_utils import run_bass_kernel_spmd
from concourse.masks import make_identity

# Route every activation (Copy/Ln/Exp) to the one act-func table that holds
# all three ("natural_log_exp_and_others"), so the kernel issues a single
# LoadActFuncSet at startup instead of three swaps on the critical path.
# Only the chooser's view is filtered; table ids still index act_info.json.
import concourse.hw_specs as _hw_specs


def _patch_act_tables():
    if getattr(_hw_specs, "_crossattn_patched", False):
        return
    orig = _hw_specs.get_activation_tables
    import functools

    @functools.cache
    def patched(arch):
        t = dict(orig(arch))
        key = "natural_log_exp_and_others"
        if key in t:
            shared = t[key]
            t = {n: (fns if n == key else fns - shared) for n, fns in t.items()}
        return t

    _hw_specs.get_activation_tables = patched
    bacc.get_activation_tables = patched
    _hw_specs._crossattn_patched = True


_patch_act_tables()

P = 128
N = 1024            # sequence length
E = 64              # embedding channels
CH = 512            # attention channels
B = 8               # kv batches
NCORES = 8
X = N // P          # 8 rows per partition in flat layout (n = 8p + x)
O = CH // P         # 4 c-chunks (c = 4p + o in flat weight layout)
EPS = 1e-5
NELEM = float(CH * B * CH)
CTXS = 2.0 ** -12   # U staged as U*2^-12; Wo scaled by 2^12/Z (fp16 range)

F32 = mybir.dt.float32
F16 = mybir.dt.float16
Exp = mybir.ActivationFunctionType.Exp
Ln = mybir.ActivationFunctionType.Ln
Mult = mybir.AluOpType.mult
Sub = mybir.AluOpType.subtract
Add = mybir.AluOpType.add
AxX = mybir.AxisListType.X


def build_kernel() -> bass.Bass:
    nc = bacc.Bacc("TRN2", debug=False, num_devices=NCORES)

    emb_l = nc.dram_tensor("emb_l", [N, E], F32, kind="ExternalInput")
    emb_u = nc.dram_tensor("emb_u", [B, N, E], F32, kind="ExternalInput")
    Wq = nc.dram_tensor("Wq", [CH, E], F32, kind="ExternalInput")
    Wk = nc.dram_tensor("Wk", [CH, E], F32, kind="ExternalInput")
    Wv = nc.dram_tensor("Wv", [CH, E], F32, kind="ExternalInput")
    Wo = nc.dram_tensor("Wo", [E, CH], F32, kind="ExternalInput")
    out = nc.dram_tensor("out", [N, E], F32, kind="ExternalOutput")

    with tile.TileContext(nc) as tc:
        _body(tc, emb_l, emb_u, Wq, Wk, Wv, Wo, out)
    nc.compile()
    return nc


def _body(tc, emb_l, emb_u, Wq, Wk, Wv, Wo, out):
    nc = tc.nc

    with ExitStack() as ctx:
        const = ctx.enter_context(tc.tile_pool(name="const", bufs=1))
        wpool = ctx.enter_context(tc.tile_pool(name="wpool", bufs=1))
        stream = ctx.enter_context(tc.tile_pool(name="stream", bufs=3))
        small = ctx.enter_context(tc.tile_pool(name="small", bufs=1))
        psum = ctx.enter_context(tc.tile_pool(name="psum", bufs=1, space="PSUM"))

        # PSUM (8 banks): tag "s2" 2 slots x 1 bank (smalls / early-U / z ...)
        #                 tag "gh" 1 slot x 2 banks (G then H')
        #                 tag "big" 2 slots x 2 banks (prep / euT-tp / sT / U)

        # ---- input DMA train: one queue, explicit order; flat layouts ----
        el_nat = stream.tile([P, X, E], F32, tag="el_nat", bufs=1, name="el_nat")
        nc.sync.dma_start(el_nat[:], emb_l.rearrange("(p x) e -> p x e", p=P))
        wq_nat = stream.tile([P, O, E], F32, tag="w_nat", bufs=2, name="wq_nat")
        nc.sync.dma_start(wq_nat[:], Wq.rearrange("(p o) e -> p o e", p=P))
        wk_nat = stream.tile([P, O, E], F32, tag="w_nat", bufs=2, name="wk_nat")
        nc.sync.dma_start(wk_nat[:], Wk.rearrange("(p o) e -> p o e", p=P))
        eu_nat = []
        for b in range(B):
            t = stream.tile([P, X, E], F32, tag="eu_nat", bufs=3,
                            name=f"eu_nat{b}")
            nc.sync.dma_start(t[:], emb_u[b].rearrange("(p x) e -> p x e", p=P))
            eu_nat.append(t)
        wv_nat = stream.tile([P, O, E], F32, tag="wv_nat", bufs=1, name="wv_nat")
        nc.sync.dma_start(wv_nat[:], Wv.rearrange("(p o) e -> p o e", p=P))
        wo_nat = stream.tile([E, CH], F32, tag="wo_nat", bufs=1, name="wo_nat")
        nc.sync.dma_start(wo_nat[:], Wo[:, :])  # host pre-permuted columns

        # ---- constants ----
        ident = const.tile([P, P], F32)
        make_identity(nc, ident)
        ident16 = const.tile([P, P], F16)
        nc.vector.tensor_copy(ident16[:], ident[:])
        ones16b = const.tile([B, 1], F16)
        nc.vector.memset(ones16b, 1.0)
        ones_f = const.tile([P, 1], F32)
        nc.vector.memset(ones_f, 1.0)
        ones_row = const.tile([1, P], F32)
        nc.vector.memset(ones_row, 1.0)
        eps_t = small.tile([1, 1], F32, name="eps_t")
        nc.vector.memset(eps_t[:], EPS)
        # [0 | I64] selection at base partition 0 (for b-pair H repacking);
        # SBUF->SBUF DMA is the partition shifter
        sel_hi = const.tile([E, P], F16)
        nc.sync.dma_start(sel_hi[:], ident16[E:P, :])

        # ---- weight prep; pass-A tiles ----
        eu16 = wpool.tile([P, B, X, E], F16, name="eu16")
        m16 = wpool.tile([E, B, E], F16, name="m16")
        G_sb = wpool.tile([E, B, CH], F16, name="G_sb")
        grs = small.tile([E, B], F32, name="grs")       # rowsum(G[b])
        euTpp = wpool.tile([P, 4, N], F16, name="euTpp")  # [(par,e), g, n]
        ps_et = {}

        el16 = wpool.tile([P, X, E], F16, name="el16")
        nc.vector.tensor_copy(el16[:], el_nat[:])
        w16q = wpool.tile([P, O, E], F16, name="w16q")
        nc.vector.tensor_copy(w16q[:], wq_nat[:])
        # b0's convert goes ahead of the prep-finisher copies so the per-b
        # pipeline starts as soon as eu[0] lands (engines run in order)
        nc.vector.tensor_copy(eu16[:, 0, :, :], eu_nat[0][:])
        w16k = wpool.tile([P, O, E], F16, name="w16k")
        nc.gpsimd.tensor_copy(w16k[:], wk_nat[:])

        wT = {}
        for wname, wsrc, eng in (("q", w16q, nc.vector), ("k", w16k, nc.scalar)):
            ps_wt = psum.tile([E, CH], F16, tag="b2", bufs=2,
                              name=f"ps_wt_{wname}")
            for o in range(O):
                nc.tensor.transpose(ps_wt[:, o * P:(o + 1) * P], wsrc[:, o, :],
                                    ident16)
            wt = wpool.tile([E, CH], F16, name=f"wT_{wname}")
            if eng is nc.scalar:
                nc.scalar.copy(wt[:], ps_wt[:])
            else:
                eng.tensor_copy(wt[:], ps_wt[:])
            wT[wname] = wt

        # row-sums of Wk^T over c (exact sum(s) via G row-sums)
        wrk = small.tile([E, 1], F32, name="wrk")
        nc.vector.reduce_sum(wrk[:], wT["k"][:], axis=AxX)

        # Q2 = Wq^T Wq (fp16 stationary), K2 = Wk^T Wk (f32 for stats mul)
        q2_16 = wpool.tile([E, E], F16, name="q2_16")
        k2_sb = wpool.tile([E, E], F32, name="k2_sb")
        for wsrc, dst, eng in ((w16q, q2_16, nc.vector),
                               (w16k, k2_sb, nc.vector)):
            ps_w2m = psum.tile([E, E], F32, tag="r1", bufs=1, name="ps_w2m")
            for o in range(O):
                nc.tensor.matmul(ps_w2m[:], wsrc[:, o, :], wsrc[:, o, :],
                                 start=(o == 0), stop=(o == O - 1))
            eng.tensor_copy(dst[:], ps_w2m[:])

        # [Wv | 1] stationary: H' row 64 = per-batch softmax denominator Z_b
        w_aug = wpool.tile([P, O, E + 1], F16, name="w_aug")
        nc.gpsimd.tensor_copy(w_aug[:, :, 0:E], wv_nat[:])
        nc.gpsimd.memset(w_aug[:, :, E:E + 1], 1.0)
        # [Wv | 0] / [0 | Wv] stationaries: direct partition-packed H' for
        # the last b-pair (each matmul writes all 128 partitions, so the
        # pair accumulates as one PSUM group)
        w_pp = wpool.tile([P, O, 2, P], F16, name="w_pp")
        nc.gpsimd.memset(w_pp[:], 0.0)
        nc.gpsimd.tensor_copy(w_pp[:, :, 0, 0:E], wv_nat[:])
        nc.gpsimd.tensor_copy(w_pp[:, :, 1, E:P], wv_nat[:])

        # ---- pass A: per kv batch -> M, G, euT (b-pair partition-packed);
        # stats matmuls (p1 = Q2 M, p3sum += M^T p1) run per b-half ----
        p1_16 = stream.tile([E, B, E], F16, tag="p1_16", bufs=1, name="p1_16")
        ps_p3 = None

        for b in range(B):
            g, par = b // 2, b % 2
            if b > 0:
                cveng = nc.vector if par == 0 else nc.gpsimd
                cveng.tensor_copy(eu16[:, b, :, :], eu_nat[b][:])

            # M[b] = emb_l^T @ emb_u[b]
            ps_mp = psum.tile([E, 2, E], F32, tag="s1", bufs=1, name="ps_mp")
            for x in range(X):
                nc.tensor.matmul(ps_mp[:, 0, :], el16[:, x, :],
                                 eu16[:, b, x, :],
                                 start=(x == 0), stop=(x == X - 1))
            nc.vector.tensor_copy(m16[:, b, :], ps_mp[:, 0, :])

            # G[b] = M^T Wq^T; row-sums accumulated during the copy
            ps_G = psum.tile([E, CH], F32, tag="h1", bufs=2, name="ps_G")
            nc.tensor.matmul(ps_G[:], m16[:, b, :], wT["q"][:],
                             start=True, stop=True)
            nc.vector.tensor_scalar(G_sb[:, b, :], ps_G[:], 1.0, None, Mult,
                                    Add, accum_out=grs[:, b:b + 1])

            # emb_u[b]^T at partition offset 64*par (b-pair packing)
            if par == 0:
                ps_et[g] = psum.tile([P, N], F16, tag="b2", bufs=2,
                                     name=f"ps_et{g}")
            for x in range(X):
                nc.tensor.transpose(
                    ps_et[g][par * E:(par + 1) * E, x * P:(x + 1) * P],
                    eu16[:, b, x, :], ident16)
            if par == 1:
                if g < 3:
                    nc.scalar.copy(euTpp[:, g, :], ps_et[g][:])
                else:
                    nc.vector.tensor_copy(euTpp[:, g, :], ps_et[g][:])

            if b % 4 == 3:  # stats matmuls for this half of the batches
                h = b // 4
                sl = slice(4 * h, 4 * h + 4)
                ps_p1 = psum.tile([E, 4, E], F32, tag="s1", bufs=1,
                                  name="ps_p1")
                nc.tensor.matmul(ps_p1[:], q2_16[:], m16[:, sl, :],
                                 start=True, stop=True)
                nc.vector.tensor_copy(p1_16[:, sl, :], ps_p1[:])
                ps_p3 = psum.tile([E, E], F32, tag="r1", bufs=1, name="ps_p3")
                for bb in range(4 * h, 4 * h + 4):
                    nc.tensor.matmul(ps_p3[:], m16[:, bb, :], p1_16[:, bb, :],
                                     start=(bb % 4 == 0), stop=(bb % 4 == 3))
                if h == 0:
                    p3a_sb = stream.tile([E, E], F32, tag="p3a", bufs=1,
                                         name="p3a_sb")
                    nc.vector.tensor_copy(p3a_sb[:], ps_p3[:])

        p3t = stream.tile([E, E], F32, tag="p3t", bufs=1, name="p3t")
        nc.vector.tensor_tensor(p3t[:], ps_p3[:], p3a_sb[:], Add)
        scr2 = stream.tile([E, E], F32, tag="scr2", bufs=1, name="scr2")
        nc.vector.tensor_tensor(scr2[:], p3t[:], k2_sb[:], Mult)

        # ---- instance-norm stats -> exp scale/bias ----
        psums2 = small.tile([E, 2], F32, name="psums2")
        grsum = small.tile([E, 1], F32, name="grsum")
        nc.vector.reduce_sum(grsum[:], grs[:], axis=AxX)
        nc.vector.tensor_scalar(psums2[:, 0:1], grsum[:], wrk[:, 0:1],
                                None, Mult)
        nc.vector.reduce_sum(psums2[:, 1:2], scr2[:], axis=AxX)
        ps_gm = psum.tile([1, 2], F32, tag="s1", bufs=1, name="ps_gm")
        nc.tensor.matmul(ps_gm[:], ones_f[0:E, :], psums2[:],
                         start=True, stop=True)
        gm = small.tile([1, 2], F32, name="gm")
        nc.vector.tensor_scalar_mul(gm[:], ps_gm[:], 1.0 / NELEM)
        nvar = small.tile([1, 1], F32, name="nvar")  # mu^2 - E[s^2] = -var
        nc.vector.scalar_tensor_tensor(nvar[:], gm[:, 0:1], gm[:, 0:1],
                                       gm[:, 1:2], Mult, Sub)
        sc = small.tile([1, 2], F32, name="sc")
        lnv = small.tile([1, 1], F32, name="lnv")
        nc.scalar.activation(lnv[:], nvar[:], Ln, bias=eps_t[:], scale=-1.0)
        nc.scalar.activation(sc[:, 0:1], lnv[:], Exp, scale=-0.5)
        nc.vector.tensor_scalar(sc[:, 1:2], gm[:, 0:1], sc[:, 0:1], -1.0,
                                Mult, Mult)
        ps_bc = psum.tile([P, 2], F32, tag="s1", bufs=1, name="ps_bc")
        nc.tensor.matmul(ps_bc[:], ones_row[:], sc[:], start=True, stop=True)
        sb_b = small.tile([P, 2], F32, name="sb_b")
        nc.vector.tensor_copy(sb_b[:], ps_bc[:])

        # ---- pass 0a: sT -> exp(fp16) -> H' (incl. Z_b row via w_aug) ----
        # software-pipelined: sT(g+1) is emitted before H'(g) so the PE
        # stream never stalls waiting on exp(g), and ACT runs back-to-back.
        eT_sb = wpool.tile([P, O, B, CH], F16, name="eT_sb")
        H_sb = wpool.tile([E + 1, B, CH], F16, name="H_sb")
        H2pp = wpool.tile([P, 4, CH], F16, name="H2pp")  # [(par,e), g, c]
        ctx_bf = wpool.tile([P, O, N], F16, name="ctx_bf")
        ps_U00 = psum.tile([P, CH], F32, tag="s1", bufs=1, name="ps_U00")

        def emit_sT(g):
            for cp in range(O):
                ps_sT = psum.tile([P, 2, CH], F32, tag="b2", bufs=2,
                                  name="ps_sT")
                for bm in range(2):
                    nc.tensor.matmul(ps_sT[:, bm, :],
                                     wT["k"][:, cp * P:(cp + 1) * P],
                                     G_sb[:, 2 * g + bm, :],
                                     start=True, stop=True)
                nc.scalar.activation(eT_sb[:, cp, 2 * g:2 * g + 2, :],
                                     ps_sT[:], Exp,
                                     bias=sb_b[:, 1:2], scale=sb_b[:, 0:1])

        zz = wpool.tile([6, CH], F16, name="zz")      # Z_b rows, groups 0-2
        zz3 = wpool.tile([2, CH], F16, name="zz3")    # Z_b rows, group 3

        def emit_H(g):
            ps_H = [psum.tile([E + 1, CH], F32, tag="h1", bufs=2,
                              name=f"ps_H{g}{bm}") for bm in range(2)]
            for cp in range(O):
                for bm in range(2):
                    nc.tensor.matmul(ps_H[bm][:], w_aug[:, cp, :],
                                     eT_sb[:, cp, 2 * g + bm, :],
                                     start=(cp == 0), stop=(cp == O - 1))
            for bm in range(2):
                nc.vector.tensor_copy(H_sb[:, 2 * g + bm, :], ps_H[bm][:])
            # Z_b rows to partitions while the rest of pass 0a runs
            nc.scalar.dma_start(zz[2 * g:2 * g + 2, :],
                                H_sb[E:E + 1, 2 * g:2 * g + 2, :])
            # repack the b-pair along partitions: H2pp[:, g] = [H[2g]; H[2g+1]]
            ps_pp = psum.tile([P, CH], F32, tag="r1", bufs=1, name="ps_pp")
            nc.tensor.matmul(ps_pp[:], ident16[0:E, :], H_sb[0:E, 2 * g, :],
                             start=True, stop=False)
            nc.tensor.matmul(ps_pp[:], sel_hi[:], H_sb[0:E, 2 * g + 1, :],
                             start=False, stop=True)
            nc.vector.tensor_copy(H2pp[:, g, :], ps_pp[:])
            # early U tile (cc=0, ns=0) rides in the pass-0a PE slack
            nc.tensor.matmul(ps_U00[:], H2pp[:, g, 0:P], euTpp[:, g, 0:CH],
                             start=(g == 0), stop=(g == 3))

        def emit_H3():
            # last group: H' is computed directly partition-packed (and, in
            # parallel, in the [65 x .] Z-row form), interleaved per score
            # chunk so PE stays busy (and warm) through the last exps.
            ps_pp = psum.tile([P, CH], F32, tag="r1", bufs=1, name="ps_pp3")
            ps_H = [psum.tile([E + 1, CH], F32, tag="h1", bufs=2,
                              name=f"ps_H3{bm}") for bm in range(2)]
            for cp in range(O):
                for par in range(2):
                    nc.tensor.matmul(ps_pp[:], w_pp[:, cp, par, :],
                                     eT_sb[:, cp, 6 + par, :],
                                     start=(cp == 0 and par == 0),
                                     stop=(cp == O - 1 and par == 1))
                for bm in range(2):
                    nc.tensor.matmul(ps_H[bm][:], w_aug[:, cp, :],
                                     eT_sb[:, cp, 6 + bm, :],
                                     start=(cp == 0), stop=(cp == O - 1))
            nc.vector.tensor_copy(H2pp[:, 3, :], ps_pp[:])
            for bm in range(2):
                nc.vector.tensor_copy(H_sb[:, 6 + bm, :], ps_H[bm][:])
            nc.scalar.dma_start(zz3[:], H_sb[E:E + 1, 6:8, :])
            nc.tensor.matmul(ps_U00[:], H2pp[:, 3, 0:P], euTpp[:, 3, 0:CH],
                             start=False, stop=True)

        emit_sT(0)
        emit_sT(1)
        emit_H(0)
        emit_sT(2)
        emit_H(1)

        # Wo^T prep rides the pass-0a PE/Pool slack (inputs land ~10us)
        wo16 = wpool.tile([E, CH], F16, name="wo16")
        nc.gpsimd.tensor_copy(wo16[:], wo_nat[:])
        ps_wo = psum.tile([P, O, E], F16, tag="h1", bufs=2, name="ps_wo")
        for o in range(O):
            nc.tensor.transpose(ps_wo[:, o, :], wo16[:, o * P:(o + 1) * P],
                                ident16[0:E, 0:E])
        woT = wpool.tile([P, O, E], F16, name="woT")
        nc.scalar.copy(woT[:], ps_wo[:])

        emit_sT(3)
        emit_H(2)
        emit_H3()

        # ---- Z = sum_b Z_b -> 4096/Z folded into Wo^T ----
        ps_z = psum.tile([1, CH], F32, tag="r1", bufs=1, name="ps_z")
        nc.tensor.matmul(ps_z[:], ones16b[0:6, :], zz[:],
                         start=True, stop=False)
        nc.tensor.matmul(ps_z[:], ones16b[0:2, :], zz3[:],
                         start=False, stop=True)
        z_sb = small.tile([1, CH], F32, name="z_sb")
        nc.vector.tensor_copy(z_sb[:], ps_z[:])
        ps_zt = psum.tile([P, O], F32, tag="r1", bufs=1, name="ps_zt")
        for o in range(O):
            nc.tensor.matmul(ps_zt[:, o:o + 1], z_sb[:, o * P:(o + 1) * P],
                             ident[0:1, 0:1],
                             start=(o == 0), stop=(o == O - 1))
        zfin = small.tile([P, O], F32, name="zfin")
        nc.vector.reciprocal(zfin[:], ps_zt[:])
        woT2 = wpool.tile([P, O, E], F16, name="woT2")
        for o in range(O):
            nc.vector.tensor_scalar(woT2[:, o, :], woT[:, o, :],
                                    zfin[:, o:o + 1], 1.0 / CTXS, Mult, Mult)

        # ---- U = H2^T @ euT (K = 128, b-pairs packed); out^T per n-half --
        oT16 = wpool.tile([E, 2, CH], F16, name="oT16")

        def emit_U(ns, cc):
            if ns == 0 and cc == 0:
                nc.vector.tensor_scalar_mul(ctx_bf[:, 0, 0:CH], ps_U00[:],
                                            CTXS)
                return
            ps_U = psum.tile([P, CH], F32, tag="b2", bufs=2, name="ps_U")
            for g in range(4):
                nc.tensor.matmul(ps_U[:], H2pp[:, g, cc * P:(cc + 1) * P],
                                 euTpp[:, g, ns * CH:(ns + 1) * CH],
                                 start=(g == 0), stop=(g == 3))
            if (cc + ns) % 2 == 0:
                nc.vector.tensor_scalar_mul(
                    ctx_bf[:, cc, ns * CH:(ns + 1) * CH], ps_U[:], CTXS)
            else:
                nc.scalar.mul(ctx_bf[:, cc, ns * CH:(ns + 1) * CH],
                              ps_U[:], CTXS)

        def emit_out(ns):
            ps_oT = psum.tile([E, CH], F32, tag="h1", bufs=2, name="ps_oT")
            for cc in range(O):
                nc.tensor.matmul(ps_oT[:], woT2[:, cc, :],
                                 ctx_bf[:, cc, ns * CH:(ns + 1) * CH],
                                 start=(cc == 0), stop=(cc == O - 1))
            nc.scalar.copy(oT16[:, ns, :], ps_oT[:])
            ps_out = psum.tile([P, O, E], F16, tag="s1", bufs=1, name="ps_out")
            for k in range(O):
                nc.tensor.transpose(ps_out[:, k, :],
                                    oT16[:, ns, k * P:(k + 1) * P],
                                    ident16[0:E, 0:E])
            out_sb = stream.tile([P, O, E], F32, tag="out_sb", bufs=2,
                                 name="out_sb")
            nc.vector.tensor_copy(out_sb[:], ps_out[:])
            nc.scalar.dma_start(
                out.rearrange("(p x) e -> p x e", p=P)[:, ns * O:(ns + 1) * O, :],
                out_sb[:])

        for cc in range(O):
            emit_U(0, cc)
        emit_U(1, 0)        # keep PE fed while ctx(0, cc3) stages
        emit_out(0)
        for cc in range(1, O):
            emit_U(1, cc)
        emit_out(1)


_NC_CACHE = None


def _get_nc():
    global _NC_CACHE
    if _NC_CACHE is None:
        _NC_CACHE = build_kernel()
    return _NC_CACHE


def kernel(**inputs) -> np.ndarray:
    emb = np.ascontiguousarray(np.asarray(inputs["emb"], dtype=np.float32))
    Wq = np.ascontiguousarray(np.asarray(inputs["Wq"], dtype=np.float32))
    Wk = np.ascontiguousarray(np.asarray(inputs["Wk"], dtype=np.float32))
    Wv = np.ascontiguousarray(np.asarray(inputs["Wv"], dtype=np.float32))
    Wo = np.ascontiguousarray(np.asarray(inputs["Wo"], dtype=np.float32))
    # permute Wo columns into the kernel's flat c-order (c = 4p + o -> col
    # o*128 + p) so natural-chunk PE transposes yield the matching Wo^T
    Wo_perm = np.ascontiguousarray(
        Wo.reshape(E, P, O).transpose(0, 2, 1).reshape(E, CH))

    emb_u = np.ascontiguousarray(emb[:B])      # replicated K/V source
    in_maps = []
    for core in range(NCORES):
        in_maps.append({
            "emb_l": np.ascontiguousarray(emb[B + core]),
            "emb_u": emb_u,
            "Wq": Wq, "Wk": Wk, "Wv": Wv, "Wo": Wo_perm,
        })

    nc = _get_nc()
    res = run_bass_kernel_spmd(nc, in_maps, core_ids=list(range(NCORES)))
    out = np.stack([res.results[c]["out"] for c in range(NCORES)], axis=0)
    return out.astype(np.float32)


if __name__ == "__main__":
    nc = build_kernel()
    print("built ok")


# revision 41
# speedup vs baseline: 1.4552x; 1.0177x over previous
"""Trainium2 Bass kernel for CrossAttnMem (q-augmented cross attention with
a shared K/V memory bank, InstanceNorm'd scores, softmax over the bank axis).

Sharding: 8 query batches over 8 cores; each core holds its query slice plus
the full emb_u (replicated) and computes its output slice locally.

Rank-64 reassociation (per kv batch b):
    M[b]   = emb_l^T @ emb_u[b]                  [64, 64]   (K = n)
    G[b]   = M[b]^T @ Wq^T                       [64, CH]
    sT[b]  = Wk @ G[b]                           [CH, CH]   (d-major scores)
    stats:   sum(s) = <wrk, sum_b rowsum(G[b])>,
             sum(s^2) = <K2, sum_b M^T Q2 M>     (exact InstanceNorm stats)
    eT[b]  = exp(s/sigma - mu/sigma)  (fp16; fp8 tested: softmax too
             concentrated after norm, quant noise -> ~3e-2 rel err)
    H'[b]  = [Wv | 1]^T @ eT[b]                  [65, CH]  (row 64 = Z_b)
    Z      = ones^T @ (Z_b rows gathered to partitions via SBUF DMA)
    U      = H2^T-slices @ euT (b-pairs packed along K=128)   [CH, N]
    out^T  = (Wo*4096/Z)^T @ (U/4096)            [64, N] -> transpose -> out

Schedule notes: single-queue DMA train (flat 2KB/partition layouts, row
n = 8p + x; weight rows c = 4p + o; Wo columns pre-permuted on host to
match); pass 0a is software-pipelined (sT(g+1) before H'(g)) so the 16 exp
activations run back-to-back on ACT; emb_u^T and H are partition-packed in
b-pairs (via transpose offsets / identity-selection matmuls) so the U
contraction runs at full K=128; one U tile accumulates inside pass 0a; a
single activation table (ln+exp+copy) is loaded once at startup.
"""

import sys
from contextlib import ExitStack

import numpy as np

try:
    import concourse.bass as bass
except ImportError:  # pragma: no cover
    sys.path.insert(0, "/opt/trn_rl_repo")
    import concourse.bass as bass

import concourse.tile as tile
from concourse import bacc, mybir
from concourse.bass_utils import run_bass_kernel_spmd
from concourse.masks import make_identity

# Route every activation (Copy/Ln/Exp) to the one act-func table that holds
# all three ("natural_log_exp_and_others"), so the kernel issues a single
# LoadActFuncSet at startup instead of three swaps on the critical path.
# Only the chooser's view is filtered; table ids still index act_info.json.
import concourse.hw_specs as _hw_specs


def _patch_act_tables():
    if getattr(_hw_specs, "_crossattn_patched", False):
        return
    orig = _hw_specs.get_activation_tables
    import functools

    @functools.cache
    def patched(arch):
        t = dict(orig(arch))
        key = "natural_log_exp_and_others"
        if key in t:
            shared = t[key]
            t = {n: (fns if n == key else fns - shared) for n, fns in t.items()}
        return t

    _hw_specs.get_activation_tables = patched
    bacc.get_activation_tables = patched
    _hw_specs._crossattn_patched = True


_patch_act_tables()

P = 128
N = 1024            # sequence length
E = 64              # embedding channels
CH = 512            # attention channels
B = 8               # kv batches
NCORES = 8
X = N // P          # 8 rows per partition in flat layout (n = 8p + x)
O = CH // P         # 4 c-chunks (c = 4p + o in flat weight layout)
EPS = 1e-5
NELEM = float(CH * B * CH)
CTXS = 2.0 ** -12   # U staged as U*2^-12; Wo scaled by 2^12/Z (fp16 range)

F32 = mybir.dt.float32
F16 = mybir.dt.float16
Exp = mybir.ActivationFunctionType.Exp
Ln = mybir.ActivationFunctionType.Ln
Mult = mybir.AluOpType.mult
Sub = mybir.AluOpType.subtract
Add = mybir.AluOpType.add
AxX = mybir.AxisListType.X


def build_kernel() -> bass.Bass:
    nc = bacc.Bacc("TRN2", debug=False, num_devices=NCORES)

    emb_l = nc.dram_tensor("emb_l", [N, E], F32, kind="ExternalInput")
    emb_u = nc.dram_tensor("emb_u", [B, N, E], F32, kind="ExternalInput")
    Wq = nc.dram_tensor("Wq", [CH, E], F32, kind="ExternalInput")
    Wk = nc.dram_tensor("Wk", [CH, E], F32, kind="ExternalInput")
    Wv = nc.dram_tensor("Wv", [CH, E], F32, kind="ExternalInput")
    Wo = nc.dram_tensor("Wo", [E, CH], F32, kind="ExternalInput")
    out = nc.dram_tensor("out", [N, E], F32, kind="ExternalOutput")

    with tile.TileContext(nc) as tc:
        _body(tc, emb_l, emb_u, Wq, Wk, Wv, Wo, out)
    nc.compile()
    return nc


def _body(tc, emb_l, emb_u, Wq, Wk, Wv, Wo, out):
    nc = tc.nc

    with ExitStack() as ctx:
        const = ctx.enter_context(tc.tile_pool(name="const", bufs=1))
        wpool = ctx.enter_context(tc.tile_pool(name="wpool", bufs=1))
        stream = ctx.enter_context(tc.tile_pool(name="stream", bufs=3))
        small = ctx.enter_context(tc.tile_pool(name="small", bufs=1))
        psum = ctx.enter_context(tc.tile_pool(name="psum", bufs=1, space="PSUM"))

        # PSUM (8 banks): tag "s2" 2 slots x 1 bank (smalls / early-U / z ...)
        #                 tag "gh" 1 slot x 2 banks (G then H')
        #                 tag "big" 2 slots x 2 banks (prep / euT-tp / sT / U)

        # ---- input DMA train: one queue, explicit order; flat layouts ----
        el_nat = stream.tile([P, X, E], F32, tag="el_nat", bufs=1, name="el_nat")
        nc.sync.dma_start(el_nat[:], emb_l.rearrange("(p x) e -> p x e", p=P))
        wq_nat = stream.tile([P, O, E], F32, tag="w_nat", bufs=2, name="wq_nat")
        nc.sync.dma_start(wq_nat[:], Wq.rearrange("(p o) e -> p o e", p=P))
        wk_nat = stream.tile([P, O, E], F32, tag="w_nat", bufs=2, name="wk_nat")
        nc.sync.dma_start(wk_nat[:], Wk.rearrange("(p o) e -> p o e", p=P))
        eu_nat = []
        for b in range(B):
            t = stream.tile([P, X, E], F32, tag="eu_nat", bufs=3,
                            name=f"eu_nat{b}")
            nc.sync.dma_start(t[:], emb_u[b].rearrange("(p x) e -> p x e", p=P))
            eu_nat.append(t)
        wv_nat = stream.tile([P, O, E], F32, tag="wv_nat", bufs=1, name="wv_nat")
        nc.sync.dma_start(wv_nat[:], Wv.rearrange("(p o) e -> p o e", p=P))
        wo_nat = stream.tile([E, CH], F32, tag="wo_nat", bufs=1, name="wo_nat")
        nc.sync.dma_start(wo_nat[:], Wo[:, :])  # host pre-permuted columns

        # ---- constants ----
        ident = const.tile([P, P], F32)
        make_identity(nc, ident)
        ident16 = const.tile([P, P], F16)
        nc.vector.tensor_copy(ident16[:], ident[:])
        ones16b = const.tile([B, 1], F16)
        nc.vector.memset(ones16b, 1.0)
        ones_f = const.tile([P, 1], F32)
        nc.vector.memset(ones_f, 1.0)
        ones_row = const.tile([1, P], F32)
        nc.vector.memset(ones_row, 1.0)
        eps_t = small.tile([1, 1], F32, name="eps_t")
        nc.vector.memset(eps_t[:], EPS)
        # [0 | I64] selection at base partition 0 (for b-pair H repacking);
        # SBUF->SBUF DMA is the partition shifter
        sel_hi = const.tile([E, P], F16)
        nc.sync.dma_start(sel_hi[:], ident16[E:P, :])

        # ---- weight prep; pass-A tiles ----
        eu16 = wpool.tile([P, B, X, E], F16, name="eu16")
        m16 = wpool.tile([E, B, E], F16, name="m16")
        G_sb = wpool.tile([E, B, CH], F16, name="G_sb")
        grs = small.tile([E, B], F32, name="grs")       # rowsum(G[b])
        euTpp = wpool.tile([P, 4, N], F16, name="euTpp")  # [(par,e), g, n]
        ps_et = {}

        el16 = wpool.tile([P, X, E], F16, name="el16")
        nc.vector.tensor_copy(el16[:], el_nat[:])
        w16q = wpool.tile([P, O, E], F16, name="w16q")
        nc.vector.tensor_copy(w16q[:], wq_nat[:])
        # b0's convert goes ahead of the prep-finisher copies so the per-b
        # pipeline starts as soon as eu[0] lands (engines run in order)
        nc.gpsimd.tensor_copy(eu16[:, 0, :, :], eu_nat[0][:])
        w16k = wpool.tile([P, O, E], F16, name="w16k")
        nc.gpsimd.tensor_copy(w16k[:], wk_nat[:])

        wT = {}
        for wname, wsrc, eng in (("q", w16q, nc.vector), ("k", w16k, nc.scalar)):
            ps_wt = psum.tile([E, CH], F16, tag="b2", bufs=2,
                              name=f"ps_wt_{wname}")
            for o in range(O):
                nc.tensor.transpose(ps_wt[:, o * P:(o + 1) * P], wsrc[:, o, :],
                                    ident16)
            wt = wpool.tile([E, CH], F16, name=f"wT_{wname}")
            if eng is nc.scalar:
                nc.scalar.copy(wt[:], ps_wt[:])
            else:
                eng.tensor_copy(wt[:], ps_wt[:])
            wT[wname] = wt

        # row-sums of Wk^T over c (exact sum(s) via G row-sums)
        wrk = small.tile([E, 1], F32, name="wrk")
        nc.vector.reduce_sum(wrk[:], wT["k"][:], axis=AxX)

        # Q2 = Wq^T Wq (fp16 stationary), K2 = Wk^T Wk (f32 for stats mul)
        q2_16 = wpool.tile([E, E], F16, name="q2_16")
        k2_sb = wpool.tile([E, E], F32, name="k2_sb")
        for wsrc, dst, eng in ((w16q, q2_16, nc.vector),
                               (w16k, k2_sb, nc.vector)):
            ps_w2m = psum.tile([E, E], F32, tag="r1", bufs=1, name="ps_w2m")
            for o in range(O):
                nc.tensor.matmul(ps_w2m[:], wsrc[:, o, :], wsrc[:, o, :],
                                 start=(o == 0), stop=(o == O - 1))
            eng.tensor_copy(dst[:], ps_w2m[:])

        # [Wv | 1] stationary: H' row 64 = per-batch softmax denominator Z_b
        w_aug = wpool.tile([P, O, E + 1], F16, name="w_aug")
        nc.gpsimd.tensor_copy(w_aug[:, :, 0:E], wv_nat[:])
        nc.gpsimd.memset(w_aug[:, :, E:E + 1], 1.0)
        # [Wv | 0] / [0 | Wv] stationaries: direct partition-packed H' for
        # the last b-pair (each matmul writes all 128 partitions, so the
        # pair accumulates as one PSUM group)
        w_pp = wpool.tile([P, O, 2, P], F16, name="w_pp")
        nc.gpsimd.memset(w_pp[:], 0.0)
        nc.gpsimd.tensor_copy(w_pp[:, :, 0, 0:E], wv_nat[:])
        nc.gpsimd.tensor_copy(w_pp[:, :, 1, E:P], wv_nat[:])

        # ---- pass A: per kv batch -> M, G, euT (b-pair partition-packed);
        # stats matmuls (p1 = Q2 M, p3sum += M^T p1) run per b-half ----
        p1_16 = stream.tile([E, B, E], F16, tag="p1_16", bufs=1, name="p1_16")
        ps_p3 = None
        # one persistent M-target; b alternates halves so M(b+1) never waits
        # on m16(b)'s copy (regions are disjoint)
        ps_mp = psum.tile([E, 2, E], F32, tag="s1", bufs=1, name="ps_mp")

        for b in range(B):
            g, par = b // 2, b % 2
            if b > 0:
                nc.gpsimd.tensor_copy(eu16[:, b, :, :], eu_nat[b][:])

            # M[b] = emb_l^T @ emb_u[b]
            for x in range(X):
                nc.tensor.matmul(ps_mp[:, b % 2, :], el16[:, x, :],
                                 eu16[:, b, x, :],
                                 start=(x == 0), stop=(x == X - 1))
            nc.scalar.copy(m16[:, b, :], ps_mp[:, b % 2, :])

            # G[b] = M^T Wq^T; row-sums accumulated during the copy
            ps_G = psum.tile([E, CH], F32, tag="h1", bufs=2, name="ps_G")
            nc.tensor.matmul(ps_G[:], m16[:, b, :], wT["q"][:],
                             start=True, stop=True)
            nc.vector.tensor_scalar(G_sb[:, b, :], ps_G[:], 1.0, None, Mult,
                                    Add, accum_out=grs[:, b:b + 1])

            # emb_u[b]^T at partition offset 64*par (b-pair packing)
            if par == 0:
                ps_et[g] = psum.tile([P, N], F16, tag="b2", bufs=2,
                                     name=f"ps_et{g}")
            for x in range(X):
                nc.tensor.transpose(
                    ps_et[g][par * E:(par + 1) * E, x * P:(x + 1) * P],
                    eu16[:, b, x, :], ident16)
            if par == 1:
                if g < 3:
                    nc.scalar.copy(euTpp[:, g, :], ps_et[g][:])
                else:
                    nc.vector.tensor_copy(euTpp[:, g, :], ps_et[g][:])

            if b % 4 == 3:  # stats matmuls for this half of the batches
                h = b // 4
                sl = slice(4 * h, 4 * h + 4)
                ps_p1 = psum.tile([E, 4, E], F32, tag="r1", bufs=1,
                                  name="ps_p1")
                nc.tensor.matmul(ps_p1[:], q2_16[:], m16[:, sl, :],
                                 start=True, stop=True)
                nc.vector.tensor_copy(p1_16[:, sl, :], ps_p1[:])
                ps_p3 = psum.tile([E, E], F32, tag="r1", bufs=1, name="ps_p3")
                for bb in range(4 * h, 4 * h + 4):
                    nc.tensor.matmul(ps_p3[:], m16[:, bb, :], p1_16[:, bb, :],
                                     start=(bb % 4 == 0), stop=(bb % 4 == 3))
                if h == 0:
                    p3a_sb = stream.tile([E, E], F32, tag="p3a", bufs=1,
                                         name="p3a_sb")
                    nc.vector.tensor_copy(p3a_sb[:], ps_p3[:])

        p3t = stream.tile([E, E], F32, tag="p3t", bufs=1, name="p3t")
        nc.vector.tensor_tensor(p3t[:], ps_p3[:], p3a_sb[:], Add)
        scr2 = stream.tile([E, E], F32, tag="scr2", bufs=1, name="scr2")
        nc.vector.tensor_tensor(scr2[:], p3t[:], k2_sb[:], Mult)

        # ---- instance-norm stats -> exp scale/bias ----
        psums2 = small.tile([E, 2], F32, name="psums2")
        grsum = small.tile([E, 1], F32, name="grsum")
        nc.vector.reduce_sum(grsum[:], grs[:], axis=AxX)
        nc.vector.tensor_scalar(psums2[:, 0:1], grsum[:], wrk[:, 0:1],
                                None, Mult)
        nc.vector.reduce_sum(psums2[:, 1:2], scr2[:], axis=AxX)
        ps_gm = psum.tile([1, 2], F32, tag="s1", bufs=1, name="ps_gm")
        nc.tensor.matmul(ps_gm[:], ones_f[0:E, :], psums2[:],
                         start=True, stop=True)
        gm = small.tile([1, 2], F32, name="gm")
        nc.vector.tensor_scalar_mul(gm[:], ps_gm[:], 1.0 / NELEM)
        nvar = small.tile([1, 1], F32, name="nvar")  # mu^2 - E[s^2] = -var
        nc.vector.scalar_tensor_tensor(nvar[:], gm[:, 0:1], gm[:, 0:1],
                                       gm[:, 1:2], Mult, Sub)
        sc = small.tile([1, 2], F32, name="sc")
        lnv = small.tile([1, 1], F32, name="lnv")
        nc.scalar.activation(lnv[:], nvar[:], Ln, bias=eps_t[:], scale=-1.0)
        nc.scalar.activation(sc[:, 0:1], lnv[:], Exp, scale=-0.5)
        nc.vector.tensor_scalar(sc[:, 1:2], gm[:, 0:1], sc[:, 0:1], -1.0,
                                Mult, Mult)
        ps_bc = psum.tile([P, 2], F32, tag="s1", bufs=1, name="ps_bc")
        nc.tensor.matmul(ps_bc[:], ones_row[:], sc[:], start=True, stop=True)
        sb_b = small.tile([P, 2], F32, name="sb_b")
        nc.vector.tensor_copy(sb_b[:], ps_bc[:])

        # ---- pass 0a: sT -> exp(fp16) -> H' (incl. Z_b row via w_aug) ----
        # software-pipelined: sT(g+1) is emitted before H'(g) so the PE
        # stream never stalls waiting on exp(g), and ACT runs back-to-back.
        eT_sb = wpool.tile([P, O, B, CH], F16, name="eT_sb")
        H_sb = wpool.tile([E + 1, B, CH], F16, name="H_sb")
        H2pp = wpool.tile([P, 4, CH], F16, name="H2pp")  # [(par,e), g, c]
        ctx_bf = wpool.tile([P, O, N], F16, name="ctx_bf")
        ps_U00 = psum.tile([P, CH], F32, tag="s1", bufs=1, name="ps_U00")

        def emit_sT(g):
            for cp in range(O):
                ps_sT = psum.tile([P, 2, CH], F32, tag="b2", bufs=2,
                                  name="ps_sT")
                for bm in range(2):
                    nc.tensor.matmul(ps_sT[:, bm, :],
                                     wT["k"][:, cp * P:(cp + 1) * P],
                                     G_sb[:, 2 * g + bm, :],
                                     start=True, stop=True)
                nc.scalar.activation(eT_sb[:, cp, 2 * g:2 * g + 2, :],
                                     ps_sT[:], Exp,
                                     bias=sb_b[:, 1:2], scale=sb_b[:, 0:1])

        zz = wpool.tile([6, CH], F16, name="zz")      # Z_b rows, groups 0-2
        zz3 = wpool.tile([2, CH], F16, name="zz3")    # Z_b rows, group 3

        def emit_H(g):
            ps_H = [psum.tile([E + 1, CH], F32, tag="h1", bufs=2,
                              name=f"ps_H{g}{bm}") for bm in range(2)]
            for cp in range(O):
                for bm in range(2):
                    nc.tensor.matmul(ps_H[bm][:], w_aug[:, cp, :],
                                     eT_sb[:, cp, 2 * g + bm, :],
                                     start=(cp == 0), stop=(cp == O - 1))
            for bm in range(2):
                nc.vector.tensor_copy(H_sb[:, 2 * g + bm, :], ps_H[bm][:])
            # Z_b rows to partitions while the rest of pass 0a runs
            nc.scalar.dma_start(zz[2 * g:2 * g + 2, :],
                                H_sb[E:E + 1, 2 * g:2 * g + 2, :])
            # repack the b-pair along partitions: H2pp[:, g] = [H[2g]; H[2g+1]]
            ps_pp = psum.tile([P, CH], F32, tag="r1", bufs=1, name="ps_pp")
            nc.tensor.matmul(ps_pp[:], ident16[0:E, :], H_sb[0:E, 2 * g, :],
                             start=True, stop=False)
            nc.tensor.matmul(ps_pp[:], sel_hi[:], H_sb[0:E, 2 * g + 1, :],
                             start=False, stop=True)
            nc.vector.tensor_copy(H2pp[:, g, :], ps_pp[:])
            # early U tile (cc=0, ns=0) rides in the pass-0a PE slack
            nc.tensor.matmul(ps_U00[:], H2pp[:, g, 0:P], euTpp[:, g, 0:CH],
                             start=(g == 0), stop=(g == 3))

        def emit_H3():
            # last group: H' is computed directly partition-packed (and, in
            # parallel, in the [65 x .] Z-row form), interleaved per score
            # chunk so PE stays busy (and warm) through the last exps.
            ps_pp = psum.tile([P, CH], F32, tag="r1", bufs=1, name="ps_pp3")
            ps_H = [psum.tile([E + 1, CH], F32, tag="h1", bufs=2,
                              name=f"ps_H3{bm}") for bm in range(2)]
            for cp in range(O):
                for par in range(2):
                    nc.tensor.matmul(ps_pp[:], w_pp[:, cp, par, :],
                                     eT_sb[:, cp, 6 + par, :],
                                     start=(cp == 0 and par == 0),
                                     stop=(cp == O - 1 and par == 1))
                for bm in range(2):
                    nc.tensor.matmul(ps_H[bm][:], w_aug[:, cp, :],
                                     eT_sb[:, cp, 6 + bm, :],
                                     start=(cp == 0), stop=(cp == O - 1))
            nc.vector.tensor_copy(H2pp[:, 3, :], ps_pp[:])
            for bm in range(2):
                nc.vector.tensor_copy(H_sb[:, 6 + bm, :], ps_H[bm][:])
            nc.scalar.dma_start(zz3[:], H_sb[E:E + 1, 6:8, :])
            nc.tensor.matmul(ps_U00[:], H2pp[:, 3, 0:P], euTpp[:, 3, 0:CH],
                             start=False, stop=True)

        emit_sT(0)
        emit_sT(1)
        emit_H(0)
        emit_sT(2)
        emit_H(1)

        # Wo^T prep rides the pass-0a PE/Pool slack (inputs land ~10us)
        wo16 = wpool.tile([E, CH], F16, name="wo16")
        nc.gpsimd.tensor_copy(wo16[:], wo_nat[:])
        ps_wo = psum.tile([P, O, E], F16, tag="h1", bufs=2, name="ps_wo")
        for o in range(O):
            nc.tensor.transpose(ps_wo[:, o, :], wo16[:, o * P:(o + 1) * P],
                                ident16[0:E, 0:E])
        woT = wpool.tile([P, O, E], F16, name="woT")
        nc.scalar.copy(woT[:], ps_wo[:])

        emit_sT(3)
        emit_H(2)
        emit_H3()

        # ---- Z = sum_b Z_b -> 4096/Z folded into Wo^T ----
        ps_z = psum.tile([1, CH], F32, tag="r1", bufs=1, name="ps_z")
        nc.tensor.matmul(ps_z[:], ones16b[0:6, :], zz[:],
                         start=True, stop=False)
        nc.tensor.matmul(ps_z[:], ones16b[0:2, :], zz3[:],
                         start=False, stop=True)
        z_sb = small.tile([1, CH], F32, name="z_sb")
        nc.vector.tensor_copy(z_sb[:], ps_z[:])
        ps_zt = psum.tile([P, O], F32, tag="r1", bufs=1, name="ps_zt")
        for o in range(O):
            nc.tensor.matmul(ps_zt[:, o:o + 1], z_sb[:, o * P:(o + 1) * P],
                             ident[0:1, 0:1],
                             start=(o == 0), stop=(o == O - 1))
        zfin = small.tile([P, O], F32, name="zfin")
        nc.vector.reciprocal(zfin[:], ps_zt[:])
        woT2 = wpool.tile([P, O, E], F16, name="woT2")
        for o in range(O):
            nc.vector.tensor_scalar(woT2[:, o, :], woT[:, o, :],
                                    zfin[:, o:o + 1], 1.0 / CTXS, Mult, Mult)

        # ---- U = H2^T @ euT (K = 128, b-pairs packed); out^T per n-half --
        oT16 = wpool.tile([E, 2, CH], F16, name="oT16")

        def emit_U(ns, cc):
            if ns == 0 and cc == 0:
                nc.vector.tensor_scalar_mul(ctx_bf[:, 0, 0:CH], ps_U00[:],
                                            CTXS)
                return
            ps_U = psum.tile([P, CH], F32, tag="b2", bufs=2, name="ps_U")
            for g in range(4):
                nc.tensor.matmul(ps_U[:], H2pp[:, g, cc * P:(cc + 1) * P],
                                 euTpp[:, g, ns * CH:(ns + 1) * CH],
                                 start=(g == 0), stop=(g == 3))
            if (cc + ns) % 2 == 0:
                nc.vector.tensor_scalar_mul(
                    ctx_bf[:, cc, ns * CH:(ns + 1) * CH], ps_U[:], CTXS)
            else:
                nc.scalar.mul(ctx_bf[:, cc, ns * CH:(ns + 1) * CH],
                              ps_U[:], CTXS)

        def emit_out(ns):
            ps_oT = psum.tile([E, CH], F32, tag="h1", bufs=2, name="ps_oT")
            for cc in range(O):
                nc.tensor.matmul(ps_oT[:], woT2[:, cc, :],
                                 ctx_bf[:, cc, ns * CH:(ns + 1) * CH],
                                 start=(cc == 0), stop=(cc == O - 1))
            nc.scalar.copy(oT16[:, ns, :], ps_oT[:])
            ps_out = psum.tile([P, O, E], F16, tag="s1", bufs=1, name="ps_out")
            for k in range(O):
                nc.tensor.transpose(ps_out[:, k, :],
                                    oT16[:, ns, k * P:(k + 1) * P],
                                    ident16[0:E, 0:E])
            out_sb = stream.tile([P, O, E], F32, tag="out_sb", bufs=2,
                                 name="out_sb")
            nc.vector.tensor_copy(out_sb[:], ps_out[:])
            nc.scalar.dma_start(
                out.rearrange("(p x) e -> p x e", p=P)[:, ns * O:(ns + 1) * O, :],
                out_sb[:])

        for cc in range(O):
            emit_U(0, cc)
        emit_U(1, 0)        # keep PE fed while ctx(0, cc3) stages
        emit_out(0)
        for cc in range(1, O):
            emit_U(1, cc)
        emit_out(1)


_NC_CACHE = None


def _get_nc():
    global _NC_CACHE
    if _NC_CACHE is None:
        _NC_CACHE = build_kernel()
    return _NC_CACHE


def kernel(**inputs) -> np.ndarray:
    emb = np.ascontiguousarray(np.asarray(inputs["emb"], dtype=np.float32))
    Wq = np.ascontiguousarray(np.asarray(inputs["Wq"], dtype=np.float32))
    Wk = np.ascontiguousarray(np.asarray(inputs["Wk"], dtype=np.float32))
    Wv = np.ascontiguousarray(np.asarray(inputs["Wv"], dtype=np.float32))
    Wo = np.ascontiguousarray(np.asarray(inputs["Wo"], dtype=np.float32))
    # permute Wo columns into the kernel's flat c-order (c = 4p + o -> col
    # o*128 + p) so natural-chunk PE transposes yield the matching Wo^T
    Wo_perm = np.ascontiguousarray(
        Wo.reshape(E, P, O).transpose(0, 2, 1).reshape(E, CH))

    emb_u = np.ascontiguousarray(emb[:B])      # replicated K/V source
    in_maps = []
    for core in range(NCORES):
        in_maps.append({
            "emb_l": np.ascontiguousarray(emb[B + core]),
            "emb_u": emb_u,
            "Wq": Wq, "Wk": Wk, "Wv": Wv, "Wo": Wo_perm,
        })

    nc = _get_nc()
    res = run_bass_kernel_spmd(nc, in_maps, core_ids=list(range(NCORES)))
    out = np.stack([res.results[c]["out"] for c in range(NCORES)], axis=0)
    return out.astype(np.float32)


if __name__ == "__main__":
    nc = build_kernel()
    print("built ok")
